# revision 1
# baseline (speedup 1.0000x reference)
"""Trainium2 Bass kernel for causal multi-head attention + output projection.

Problem: B=4, T=2048, C=1024, H=16 heads (hd=64), causal softmax with
scale C**-0.5, then nn.Linear(C, C): y = attn_out @ W_out.T + b_out.

Sharding (8 cores): core = (b, head_half); b = core // 2, half = core % 2.
Each core computes 8 heads (4 head-pairs) over ALL 2048 rows of its batch
element -- every core runs the identical SPMD program (the causal loop
structure does not depend on the core id; only the data differs).  The
output projection contracts only the core's 512 channels, producing a
partial sum; the host adds the two partials per batch (replacing the
all-reduce) and adds the bias.

On-chip layout notes:
 - scoresT orientation: scores^T[k, q] = kT.T @ qT per head, so softmax
   denominators come from a ones-column appended to V (attn@V computes
   [65, q]: rows 0..63 = head dims, row 64 = sum of exp).
 - q/k/W are pre-transposed on the host (bf16), so no on-chip transposes.
 - Head pairs run as concurrent K=64 row-tiled matmuls (partitions 0-63 /
   64-127 of the PE array).
 - exp runs on ACT from PSUM with scale=C**-0.5 folded in; causal masking
   multiplies a precomputed 128x128 staircase on the diagonal blocks only.
"""

import os
import sys

for _p in ("/opt/trn_rl_repo", "/root/.axon_site/_ro/trn_rl_repo"):
    if os.path.isdir(_p) and _p not in sys.path:
        sys.path.insert(0, _p)

import numpy as np
import ml_dtypes
from contextlib import ExitStack

bf16 = ml_dtypes.bfloat16

B, T, C, H, HD = 4, 2048, 1024, 16, 64
NCORES = 8
HPC = H // 2          # heads per core = 8
PAIRS = HPC // 2      # head pairs per core = 4
CH = C // 2           # channels per core = 512
SCALE = float(C) ** -0.5

_CACHED = {}

# Tuning knobs (overridable before _get_program() for sim sweeps)
TUNE = {
    "at_bufs": 6,       # attnT SBUF pool bufs
    "sc_bufs": 2,       # scores PSUM pool bufs (tiles of 2 banks each)
    "ac_bufs": 3,       # accumulator PSUM pool bufs (1 bank each)
    "pj_bufs": 1,       # projection PSUM pool bufs (1 bank each)
    "y_bufs": 3,
    "rb_psum_mult": True,
    "pipe_depth": 2,    # tiles attn@V trails the scores/exp stream by
    "nb_delay": 4,      # tiles between a pair's reciprocals and its norm_b
    "pp_every": 4,      # emit a deferred projection chunk every N tiles
    "pj_pool": True,
    "flush_ge": 1,
    "loop_n": 1,        # >1: repeat body in one NEFF (benchmark mode)
    "level": 4,         # build truncation for phase benchmarks (4=full)
    "proj_interleave": True,
    "proj_dma": True,
    "proj_src_const": False,
    "y_on_act": False,  # projection PSUM->SBUF copies on ACT instead of DVE
}


def _patch_act_tables():
    # The act-table placement pass maps each ACT func to the lowest-id set
    # containing it, which thrashes between exp_and_others and natural_log
    # (2.7us per reload).  Strip Exp/Ln/Copy from every set except the
    # combined natural_log_exp_and_others so all our ACT work shares one
    # table load.  Set ids (dict order) are preserved.
    from concourse import bacc as _bacc
    from concourse.hw_specs import get_activation_tables as _orig

    if getattr(_bacc, "_mha_act_patch", False):
        return
    import concourse.mybir as mybir

    keep = {
        mybir.ActivationFunctionType.Exp,
        mybir.ActivationFunctionType.Ln,
        mybir.ActivationFunctionType.Copy,
    }

    def patched(arch):
        tables = {k: set(v) for k, v in _orig(arch).items()}
        if "natural_log_exp_and_others" in tables and keep <= tables[
            "natural_log_exp_and_others"
        ]:
            for name, fns in tables.items():
                if name != "natural_log_exp_and_others":
                    fns -= keep
        return tables

    _bacc.get_activation_tables = patched
    _bacc._mha_act_patch = True


def _build_program():
    import concourse.bass as bass
    import concourse.tile as tile
    import concourse.mybir as mybir
    from concourse import bacc

    _patch_act_tables()

    f32 = mybir.dt.float32
    bf = mybir.dt.bfloat16
    Exp = mybir.ActivationFunctionType.Exp
    Ln = mybir.ActivationFunctionType.Ln

    nc = bacc.Bacc()
    qT_d = nc.declare_dram_parameter("qT", [CH, T], bf, isOutput=False)
    kT_d = nc.declare_dram_parameter("kT", [CH, T], bf, isOutput=False)
    vx_d = nc.declare_dram_parameter("vx", [T, HPC * 65], bf, isOutput=False)
    wT_d = nc.declare_dram_parameter("wT", [CH, C], bf, isOutput=False)
    mk_d = nc.declare_dram_parameter("mk", [128, 256], bf, isOutput=False)
    on_d = nc.declare_dram_parameter("on", [1, 64], bf, isOutput=False)
    yp_d = nc.declare_dram_parameter("yp", [T, C], bf, isOutput=True)

    with tile.TileContext(nc) as tc, ExitStack() as ctx:
        const = ctx.enter_context(tc.tile_pool(name="const", bufs=1))

        # Persistent SBUF tensors
        qT_sb = const.tile([128, PAIRS, T], bf)      # [p, j, t]; p = pair channel
        kT_sb = const.tile([128, PAIRS, T], bf)
        v_sb = const.tile([128, 16, HPC * 65], bf)   # [p, kb, h*65+e]
        wt_sb = const.tile([128, PAIRS, C], bf)      # [p, ci, n]
        mk_sb = const.tile([128, 256], bf)
        on_sb = const.tile([128, 64], bf)            # row 64 holds ones
        aoT_sb = const.tile([128, PAIRS, T], bf)     # attn outputs, transposed

        for j in range(PAIRS):
            nc.sync.dma_start(
                qT_sb[:, j, :],
                qT_d[:].rearrange("(j p) t -> j p t", p=128)[j],
            )
            nc.sync.dma_start(
                kT_sb[:, j, :],
                kT_d[:].rearrange("(j p) t -> j p t", p=128)[j],
            )
            nc.sync.dma_start(
                wt_sb[:, j, :],
                wT_d[:].rearrange("(j p) n -> j p n", p=128)[j],
            )
        vx_r = vx_d[:].rearrange("(g kb p) e -> g p kb e", p=128, g=4)
        for g in range(4):
            nc.sync.dma_start(v_sb[:, 4 * g : 4 * (g + 1), :], vx_r[g])
        nc.sync.dma_start(mk_sb[:], mk_d[:])
        nc.sync.dma_start(on_sb[64:65, :], on_d[:])

        mk_v = mk_sb[:].rearrange("p (g o) -> p g o", g=2)

        atp = ctx.enter_context(tc.tile_pool(name="attnT", bufs=TUNE["at_bufs"]))
        sums = ctx.enter_context(tc.tile_pool(name="sums", bufs=3))
        rbsp = ctx.enter_context(tc.tile_pool(name="rbs", bufs=3))
        tmpp = ctx.enter_context(tc.tile_pool(name="tmp", bufs=3))
        ypool = ctx.enter_context(tc.tile_pool(name="yout", bufs=TUNE["y_bufs"]))
        ps_sc = ctx.enter_context(
            tc.tile_pool(name="ps_sc", bufs=TUNE["sc_bufs"], space="PSUM")
        )
        ps_ac = ctx.enter_context(
            tc.tile_pool(name="ps_ac", bufs=TUNE["ac_bufs"], space="PSUM")
        )
        ps_pj = (
            ctx.enter_context(
                tc.tile_pool(name="ps_pj", bufs=TUNE["pj_bufs"], space="PSUM")
            )
            if TUNE["pj_pool"]
            else None
        )

        def norm_b(p):
            # deferred normalize: recips via exp(-ln), broadcast across
            # partitions (K=1 matmuls), scale the SBUF copies into aoT_sb
            jj, qq0, un, r32, pid = p
            with nc.named_scope("norm"):
                rbf = sums.tile([65, 1024], bf, tag="rbf", name=f"rbf_{pid}")
                nc.scalar.activation(
                    out=rbf[64:65, :], in_=r32[64:65, :], func=Exp, scale=-1.0
                )
                rb = ps_sc.tile([128, 2, 512], f32, tag="sc", name=f"rb_{pid}")
                for h in range(2):
                    nc.tensor.matmul(
                        out=rb[0:64, h, :],
                        lhsT=on_sb[64:65, :],
                        rhs=rbf[64:65, h * 512 : (h + 1) * 512],
                        start=True,
                        stop=True,
                    )
                if TUNE["rb_psum_mult"]:
                    # un is in SBUF, so the multiplies can read the
                    # broadcast directly from PSUM (single-PSUM-operand op)
                    rb0 = rb[0:64, 0, :]
                    rb1 = rb[0:64, 1, :]
                else:
                    rb_sb = rbsp.tile([64, 1024], bf, tag="rbsb", name=f"rbsb_{pid}")
                    nc.vector.tensor_copy(rb_sb[:, :], rb[0:64, :, :])
                    rb0 = rb_sb[0:64, 0:512]
                    rb1 = rb_sb[0:64, 512:1024]
                nc.vector.tensor_mul(
                    out=aoT_sb[0:64, jj, qq0 : qq0 + 512],
                    in0=un[:, 0:512],
                    in1=rb0,
                )
                t1 = tmpp.tile([64, 512], bf, tag="t1", name=f"t1_{pid}")
                nc.vector.tensor_mul(out=t1[:, :], in0=un[:, 512:1024], in1=rb1)
                nc.sync.dma_start(aoT_sb[64:128, jj, qq0 : qq0 + 512], t1[:, :])

        def emit_proj(qc):
            # partial projection (contract the core's CH channels) for one
            # 128-row chunk
            if LEVEL < 4:
                return
            src_t = wt_sb if TUNE["proj_src_const"] else aoT_sb
            with nc.named_scope("proj"):
                y_sb = ypool.tile([128, C], bf, tag="y", name=f"y_{qc}")
                pjt = (
                    None
                    if ps_pj is not None
                    else ps_sc.tile([128, 2, 512], f32, tag="sc", name=f"pj_{qc}")
                )
                for nt in range(2):
                    if pjt is None:
                        pj = ps_pj.tile([128, 512], f32, tag="pj", name=f"pj_{qc}_{nt}")
                    else:
                        pj = pjt[:, nt, :]
                    for ci in range(PAIRS):
                        nc.tensor.matmul(
                            out=pj[:, :],
                            lhsT=src_t[:, ci, qc * 128 % src_t.shape[2] : qc * 128 % src_t.shape[2] + 128],
                            rhs=wt_sb[:, ci, nt * 512 : (nt + 1) * 512],
                            start=(ci == 0),
                            stop=(ci == PAIRS - 1),
                        )
                    if TUNE["y_on_act"]:
                        nc.scalar.copy(y_sb[:, nt * 512 : (nt + 1) * 512], pj[:, :])
                    else:
                        nc.vector.tensor_copy(
                            y_sb[:, nt * 512 : (nt + 1) * 512], pj[:, :]
                        )
                if TUNE["proj_dma"]:
                    nc.sync.dma_start(yp_d[qc * 128 : (qc + 1) * 128, :], y_sb[:, :])

        pending_proj = []

        # Optional benchmark loop: repeat the whole compute body loop_n
        # times in one NEFF (the body is idempotent) to measure HW time as
        # a wall-clock delta without per-call transfer overhead.
        loop_ctx = (
            tc.For_i(0, TUNE["loop_n"], 1) if TUNE["loop_n"] > 1 else None
        )
        if loop_ctx is not None:
            ctx.enter_context(loop_ctx)

        # Flat software-pipelined stream over (pair, kb): scores+exp lead,
        # attn@V trails by pipe_depth tiles (crossing pair boundaries), the
        # normalize broadcast trails one pair, projection trails one q-tile.
        DEPTH = TUNE["pipe_depth"]
        LEVEL = TUNE["level"]
        NB_DELAY = TUNE["nb_delay"]
        PP_EVERY = TUNE["pp_every"]
        normed = []  # (tick, norm_b payload) awaiting emission
        tick = [0]
        for qt in range(4):
            q0 = qt * 512
            nkb = 4 * qt + 4
            accs = {}
            pends = []       # (j, kb, c0, at)
            done_pairs = []  # pairs whose last attn@V has been emitted

            def pop_av():
                pj_, pkb, pc0, pat = pends.pop(0)
                if pkb == 0:
                    # free the oldest pair's acc banks before a new pair's
                    # accumulation claims slots (deadlock avoidance): with 4
                    # acc bufs, two pairs may be in flight, so only the
                    # second-oldest pending normalize must flush here
                    while len(normed) >= TUNE["flush_ge"]:
                        norm_b(normed.pop(0)[1])
                pacc = accs[pj_]
                if LEVEL >= 2:
                    with nc.named_scope("av"):
                        for h in range(2):
                            nc.tensor.matmul(
                                out=pacc[h][0:65, pc0:],
                                lhsT=v_sb[
                                    :, pkb, (2 * pj_ + h) * 65 : (2 * pj_ + h) * 65 + 65
                                ],
                                rhs=pat[:, h, pc0:],
                                start=(pkb == 0),
                                stop=(pkb == nkb - 1),
                            )
                if pkb == nkb - 1:
                    done_pairs.append(pj_)

            def flush_done():
                # norm_a (reciprocals) for any pair whose attn@V finished
                while done_pairs:
                    dj = done_pairs.pop(0)
                    dacc = accs.pop(dj)
                    if LEVEL < 3:
                        continue
                    # Release the acc PSUM banks fast: ln of the sum rows
                    # on ACT (1/s computed later as exp(-ln s); the DVE
                    # reciprocal is an iterative divide, ~8x the cost) plus
                    # DVE copies of the unnormalized outputs to SBUF.
                    with nc.named_scope("norm"):
                        r32 = sums.tile(
                            [65, 1024], f32, tag="r32", name=f"r32_{qt}_{dj}"
                        )
                        nc.scalar.activation(
                            out=r32[64:65, 0:512], in_=dacc[0][64:65, :], func=Ln
                        )
                        nc.scalar.activation(
                            out=r32[64:65, 512:1024], in_=dacc[1][64:65, :], func=Ln
                        )
                        un = sums.tile([64, 1024], bf, tag="un", name=f"un_{qt}_{dj}")
                        nc.vector.tensor_copy(un[:, 0:512], dacc[0][0:64, :])
                        nc.vector.tensor_copy(un[:, 512:1024], dacc[1][0:64, :])
                    normed.append((tick[0], (dj, q0, un, r32, f"{qt}_{dj}")))

            for j in range(PAIRS):
                accs[j] = [
                    ps_ac.tile([128, 512], f32, tag="acc", name=f"acc0_{qt}_{j}"),
                    ps_ac.tile([128, 512], f32, tag="acc", name=f"acc1_{qt}_{j}"),
                ]
                for kb in range(nkb):
                    c0 = max(0, (kb - 4 * qt) * 128)
                    sc = ps_sc.tile(
                        [128, 2, 512], f32, tag="sc", name=f"sc_{qt}_{j}_{kb}"
                    )
                    with nc.named_scope("sc"):
                        for h in range(2):
                            nc.tensor.matmul(
                                out=sc[:, h, c0:],
                                lhsT=kT_sb[
                                    h * 64 : (h + 1) * 64, j, kb * 128 : (kb + 1) * 128
                                ],
                                rhs=qT_sb[h * 64 : (h + 1) * 64, j, q0 + c0 : q0 + 512],
                                start=True,
                                stop=True,
                            )
                    at = atp.tile([128, 2, 512], bf, tag="at", name=f"at_{qt}_{j}_{kb}")
                    if LEVEL >= 1:
                        with nc.named_scope("exp"):
                            nc.scalar.activation(
                                out=at[:, :, c0:],
                                in_=sc[:, :, c0:],
                                func=Exp,
                                scale=SCALE,
                            )
                        if kb >= 4 * qt:  # diagonal block: causal staircase mask
                            with nc.named_scope("mask"):
                                nc.vector.tensor_mul(
                                    out=at[:, :, c0 : c0 + 128],
                                    in0=at[:, :, c0 : c0 + 128],
                                    in1=mk_v,
                                )
                    pends.append((j, kb, c0, at))
                    if len(pends) > DEPTH:
                        pop_av()
                        flush_done()
                    tick[0] += 1
                    # interleave deferred work from previous pairs/q-tiles
                    if normed and tick[0] - normed[0][0] >= NB_DELAY:
                        norm_b(normed.pop(0)[1])
                    if (
                        TUNE["proj_interleave"]
                        and pending_proj
                        and tick[0] % PP_EVERY == 0
                        and qt > 0
                    ):
                        emit_proj(pending_proj.pop(0))

            while pends:
                pop_av()
            flush_done()
            if qt == 3:
                while normed:
                    norm_b(normed.pop(0)[1])
            pending_proj.extend(range(4 * qt, 4 * qt + 4))
            if qt == 3:
                while pending_proj:
                    emit_proj(pending_proj.pop(0))

    nc.finalize()
    return nc


def _get_program():
    if "nc" not in _CACHED:
        _CACHED["nc"] = _build_program()
    return _CACHED["nc"]


def _prep_inputs(q, k, v, W_out):
    """Build the 8 per-core input maps (host-side shard + transpose + cast)."""
    mk = np.zeros((128, 2, 128), np.float32)
    kk = np.arange(128)[:, None]
    oo = np.arange(128)[None, :]
    mk[:, 0, :] = (kk <= oo).astype(np.float32)
    mk[:, 1, :] = mk[:, 0, :]
    mk = mk.reshape(128, 256).astype(bf16)
    ones = np.ones((1, 64), bf16)

    in_maps = []
    for core in range(NCORES):
        b, hh = core // 2, core % 2
        ch0 = hh * CH
        qT = np.ascontiguousarray(q[b].T[ch0 : ch0 + CH]).astype(bf16)
        kT = np.ascontiguousarray(k[b].T[ch0 : ch0 + CH]).astype(bf16)
        vh = v[b].reshape(T, H, HD)[:, hh * HPC : (hh + 1) * HPC, :]
        vx = np.concatenate(
            [vh.astype(np.float32), np.ones((T, HPC, 1), np.float32)], axis=2
        )
        vx = np.ascontiguousarray(vx.reshape(T, HPC * 65)).astype(bf16)
        wT = np.ascontiguousarray(W_out.T[ch0 : ch0 + CH]).astype(bf16)
        in_maps.append(
            {"qT": qT, "kT": kT, "vx": vx, "wT": wT, "mk": mk, "on": ones}
        )
    return in_maps


def _run(in_maps, trace=False):
    from concourse.bass_utils import run_bass_kernel_spmd

    nc = _get_program()
    return run_bass_kernel_spmd(
        nc, in_maps, core_ids=list(range(NCORES)), trace=trace
    )


def kernel(q, k, v, W_out, b_out, _trace=False, _return_res=False):
    q = np.asarray(q, np.float32)
    k = np.asarray(k, np.float32)
    v = np.asarray(v, np.float32)
    W_out = np.asarray(W_out, np.float32)
    b_out = np.asarray(b_out, np.float32)

    in_maps = _prep_inputs(q, k, v, W_out)
    res = _run(in_maps, trace=_trace)

    y = np.empty((B, T, C), np.float32)
    for b in range(B):
        y[b] = res.results[2 * b]["yp"].astype(np.float32) + res.results[2 * b + 1][
            "yp"
        ].astype(np.float32)
    y += b_out[None, None, :]
    if _return_res:
        return y, res
    return y



# revision 48
# speedup vs baseline: 1.5623x; 1.5623x over previous
"""Trainium2 Bass kernel for causal multi-head attention + output projection.

Problem: B=4, T=2048, C=1024, H=16 heads (hd=64), causal softmax with
scale C**-0.5, then nn.Linear(C, C): y = attn_out @ W_out.T + b_out.

Sharding (8 cores): core = (b, head_half); b = core // 2, half = core % 2.
Each core computes 8 heads (4 head-pairs) over ALL 2048 rows of its batch
element -- every core runs the identical SPMD program (the causal loop
structure does not depend on the core id; only the data differs).  The
output projection contracts only the core's 512 channels, producing a
partial sum; the host adds the two partials per batch (replacing the
all-reduce) and adds the bias.

On-chip layout notes:
 - scoresT orientation: scores^T[k, q] = kT.T @ qT per head, so softmax
   denominators come from a ones-column appended to V (attn@V computes
   [65, q]: rows 0..63 = head dims, row 64 = sum of exp).
 - q/k/W are pre-transposed on the host (bf16), so no on-chip transposes.
 - Head pairs run as K=64 row-tiled matmuls (partitions 0-63 / 64-127 of
   the PE array; distinct tile_positions let HW overlap them).
 - exp runs on ACT from PSUM with scale=C**-0.5 folded in; causal masking
   multiplies a precomputed 128x128 staircase on the diagonal blocks only
   (on GPSIMD, which is otherwise idle).
 - softmax normalization: sum rows (partition 64 of each accumulator) are
   copied to SBUF with the un-copies, DMA-gathered onto partitions 0-7 of
   a staging tile, processed with ONE batched Ln + exp(-x) on ACT per
   2-pair group, then replicated across partitions with partition_broadcast
   DMAs (no PSUM, no K=1 matmuls).
"""

import os
import sys

for _p in ("/opt/trn_rl_repo", "/root/.axon_site/_ro/trn_rl_repo"):
    if os.path.isdir(_p) and _p not in sys.path:
        sys.path.insert(0, _p)

import numpy as np
import ml_dtypes
from contextlib import ExitStack

bf16 = ml_dtypes.bfloat16

B, T, C, H, HD = 4, 2048, 1024, 16, 64
NCORES = 8
HPC = H // 2          # heads per core = 8
PAIRS = HPC // 2      # head pairs per core = 4
CH = C // 2           # channels per core = 512
SCALE = float(C) ** -0.5

_CACHED = {}

# Tuning knobs (overridable before _get_program() for sim sweeps)
TUNE = {
    "at_bufs": 6,       # attnT SBUF pool bufs
    "sc_bufs": 2,       # scores PSUM pool bufs (tiles of 2 banks each)
    "ac_bufs": 3,       # accumulator PSUM pool bufs (1 bank each)
    "pj_bufs": 1,       # projection PSUM pool bufs (1 bank each)
    "y_bufs": 3,
    "pipe_depth": 3,    # tiles attn@V trails the scores/exp stream by
    "nb_delay": 4,      # tiles between a batch's recips and its norm_b
    "pp_every": 13,     # emit a deferred projection chunk every N tiles
    "loop_n": 1,        # >1: repeat body in one NEFF (benchmark mode)
    "level": 4,         # build truncation for phase benchmarks (4=full)
    "proj_interleave": True,
    "proj_dma": True,
    "mask_gpsimd": False,  # keep Pool to one library (partition_broadcast)
    "tail_sc_proj": True,  # tail projections borrow the idle sc PSUM pool
    "tail_direct_norm": True,  # last batch: per-pair Ln/exp at partition 64
    "sums_bufs": 4,
    # partition_broadcast works in CoreSim but produces NaN through the
    # neuronxcc/PJRT path on real TRN2 -- keep both of these False
    "gather_pbcast": False,  # sum-row gather via partition_broadcast vs DMA
    "rb_pbcast": False,      # recip broadcast via partition_broadcast vs matmul
    "act_delay": 4,          # ticks before emitting a deferred batch Ln/exp
}


def _patch_act_tables():
    # The act-table placement pass maps each ACT func to the lowest-id set
    # containing it, which thrashes between exp_and_others and natural_log
    # (2.7us per reload).  Strip Exp/Ln/Copy from every set except the
    # combined natural_log_exp_and_others so all our ACT work shares one
    # table load.  Set ids (dict order) are preserved.
    from concourse import bacc as _bacc
    from concourse.hw_specs import get_activation_tables as _orig

    if getattr(_bacc, "_mha_act_patch", False):
        return
    import concourse.mybir as mybir

    keep = {
        mybir.ActivationFunctionType.Exp,
        mybir.ActivationFunctionType.Ln,
        mybir.ActivationFunctionType.Copy,
    }

    def patched(arch):
        tables = {k: set(v) for k, v in _orig(arch).items()}
        if "natural_log_exp_and_others" in tables and keep <= tables[
            "natural_log_exp_and_others"
        ]:
            for name, fns in tables.items():
                if name != "natural_log_exp_and_others":
                    fns -= keep
        return tables

    _bacc.get_activation_tables = patched
    _bacc._mha_act_patch = True


def _build_program():
    import concourse.bass as bass
    import concourse.tile as tile
    import concourse.mybir as mybir
    from concourse import bacc

    _patch_act_tables()

    f32 = mybir.dt.float32
    bf = mybir.dt.bfloat16
    Exp = mybir.ActivationFunctionType.Exp
    Ln = mybir.ActivationFunctionType.Ln

    nc = bacc.Bacc()
    qT_d = nc.declare_dram_parameter("qT", [CH, T], bf, isOutput=False)
    kT_d = nc.declare_dram_parameter("kT", [CH, T], bf, isOutput=False)
    vx_d = nc.declare_dram_parameter("vx", [T, HPC * 65], bf, isOutput=False)
    wT_d = nc.declare_dram_parameter("wT", [CH, C], bf, isOutput=False)
    mk_d = nc.declare_dram_parameter("mk", [128, 256], bf, isOutput=False)
    id_d = nc.declare_dram_parameter("id64", [64, 64], bf, isOutput=False)
    on_d = nc.declare_dram_parameter("on", [128, 64], bf, isOutput=False)
    yp_d = nc.declare_dram_parameter("yp", [T, C], bf, isOutput=True)

    with tile.TileContext(nc) as tc, ExitStack() as ctx:
        const = ctx.enter_context(tc.tile_pool(name="const", bufs=1))

        # Persistent SBUF tensors
        qT_sb = const.tile([128, PAIRS, T], bf)      # [p, j, t]; p = pair channel
        kT_sb = const.tile([128, PAIRS, T], bf)
        v_sb = const.tile([128, 16, HPC * 65], bf)   # [p, kb, h*65+e]
        wt_sb = const.tile([128, PAIRS, C], bf)      # [p, ci, n]
        mk_sb = const.tile([128, 256], bf)
        id_sb = const.tile([64, 64], bf)             # identity (partition mover)
        on_sb = const.tile([128, 64], bf)            # all-ones (bcast matmuls)
        aoT_sb = const.tile([128, PAIRS, T], bf)     # attn outputs, transposed

        # DMA issue order = earliest-needed first: mask, pair 0 q/k, the
        # first v group (rows 0-511, feeds all qt=0 attn@V), remaining q/k
        # pairs, remaining v groups, then the projection weights (needed
        # ~40 tiles in).
        qT_r = qT_d[:].rearrange("(j p) t -> j p t", p=128)
        kT_r = kT_d[:].rearrange("(j p) t -> j p t", p=128)
        wT_r = wT_d[:].rearrange("(j p) n -> j p n", p=128)
        vx_r = vx_d[:].rearrange("(g kb p) e -> g p kb e", p=128, g=4)
        # the qt=0 stream only touches the first 512 columns of q/k -- land
        # all pairs' leading halves first (kT before qT per pair: scores for
        # pair j+1 start on kT while qT streams), then the remainders
        nc.sync.dma_start(kT_sb[:, 0, 0:512], kT_r[0][:, 0:512])
        nc.sync.dma_start(qT_sb[:, 0, 0:512], qT_r[0][:, 0:512])
        nc.sync.dma_start(mk_sb[:], mk_d[:])
        nc.sync.dma_start(v_sb[:, 0:4, :], vx_r[0])
        for j in range(1, PAIRS):
            nc.sync.dma_start(kT_sb[:, j, 0:512], kT_r[j][:, 0:512])
            nc.sync.dma_start(qT_sb[:, j, 0:512], qT_r[j][:, 0:512])
        nc.sync.dma_start(on_sb[:], on_d[:])
        for j in range(PAIRS):
            nc.sync.dma_start(kT_sb[:, j, 512:T], kT_r[j][:, 512:T])
            nc.sync.dma_start(qT_sb[:, j, 512:T], qT_r[j][:, 512:T])
        for g in range(1, 4):
            nc.sync.dma_start(v_sb[:, 4 * g : 4 * (g + 1), :], vx_r[g])
        for j in range(PAIRS):
            nc.sync.dma_start(wt_sb[:, j, :], wT_r[j])
        nc.sync.dma_start(id_sb[:], id_d[:])

        mk_v = mk_sb[:].rearrange("p (g o) -> p g o", g=2)

        atp = ctx.enter_context(tc.tile_pool(name="attnT", bufs=TUNE["at_bufs"]))
        sums = ctx.enter_context(tc.tile_pool(name="sums", bufs=TUNE["sums_bufs"]))
        srowp = ctx.enter_context(tc.tile_pool(name="srow", bufs=2))
        rcpp = ctx.enter_context(tc.tile_pool(name="rcp", bufs=4))
        rbsp = ctx.enter_context(tc.tile_pool(name="rbs", bufs=4))
        tmpp = ctx.enter_context(tc.tile_pool(name="tmp", bufs=3))
        ypool = ctx.enter_context(tc.tile_pool(name="yout", bufs=TUNE["y_bufs"]))
        ps_sc = ctx.enter_context(
            tc.tile_pool(name="ps_sc", bufs=TUNE["sc_bufs"], space="PSUM")
        )
        ps_ac = ctx.enter_context(
            tc.tile_pool(name="ps_ac", bufs=TUNE["ac_bufs"], space="PSUM")
        )
        ps_pj = ctx.enter_context(
            tc.tile_pool(name="ps_pj", bufs=TUNE["pj_bufs"], space="PSUM")
        )

        LEVEL = TUNE["level"]

        def norm_b(p):
            # deferred normalize: replicate the reciprocals across partitions
            # with a partition_broadcast DMA, then scale the SBUF copies into
            # aoT_sb.  All-SBUF bf16 muls -> DVE 2x mode.  rows = [(partition,
            # free offset), ...] locating each head's reciprocal row in rcp.
            # If uhi is set (tail pairs), the h1 half already sits in PSUM at
            # partitions 64-127 (identity matmul) and the mul lands directly
            # in aoT_sb -- no cross-partition DMA on the critical path.
            jj, qq0, un, rcp, rows, uhi, pid = p
            (p0, f0), (p1, f1) = rows
            if not TUNE["rb_pbcast"]:
                # fallback: K=1 ones-matmul broadcast through a PSUM bank
                with nc.named_scope("norm"):
                    rb = ps_pj.tile([128, 512], f32, tag="pj", name=f"rb0_{pid}")
                    nc.tensor.matmul(
                        out=rb[0:64, :],
                        lhsT=on_sb[p0 : p0 + 1, :],
                        rhs=rcp[p0 : p0 + 1, f0 : f0 + 512],
                        start=True,
                        stop=True,
                        tile_position=(p0, 0),
                    )
                    nc.vector.tensor_mul(
                        out=aoT_sb[0:64, jj, qq0 : qq0 + 512],
                        in0=un[0:64, 0:512],
                        in1=rb[0:64, :],
                    )
                    rb1 = ps_pj.tile([128, 512], f32, tag="pj", name=f"rb1_{pid}")
                    if uhi is not None:
                        # tail pair: h1 already parked at partitions 64-127
                        # in SBUF -- broadcast into the same partitions and
                        # multiply straight into aoT (no cross-partition DMA)
                        nc.tensor.matmul(
                            out=rb1[64:128, :],
                            lhsT=on_sb[p1 : p1 + 1, :],
                            rhs=rcp[p1 : p1 + 1, f1 : f1 + 512],
                            start=True,
                            stop=True,
                            tile_position=(p1, 64),
                        )
                        nc.vector.tensor_mul(
                            out=aoT_sb[64:128, jj, qq0 : qq0 + 512],
                            in0=uhi[64:128, :],
                            in1=rb1[64:128, :],
                        )
                        return
                    nc.tensor.matmul(
                        out=rb1[0:64, :],
                        lhsT=on_sb[p1 : p1 + 1, :],
                        rhs=rcp[p1 : p1 + 1, f1 : f1 + 512],
                        start=True,
                        stop=True,
                        tile_position=(p1, 0),
                    )
                    t1 = tmpp.tile([64, 512], bf, tag="t1", name=f"t1_{pid}")
                    nc.vector.tensor_mul(
                        out=t1[:, :], in0=un[0:64, 512:1024], in1=rb1[0:64, :]
                    )
                    nc.sync.dma_start(
                        aoT_sb[64:128, jj, qq0 : qq0 + 512], t1[:, :]
                    )
                return
            with nc.named_scope("norm"):
                rb0 = rbsp.tile([64, 512], bf, tag="rbs", name=f"rb0_{pid}")
                nc.gpsimd.partition_broadcast(
                    rb0[:, :], rcp[p0 : p0 + 1, f0 : f0 + 512]
                )
                nc.vector.tensor_mul(
                    out=aoT_sb[0:64, jj, qq0 : qq0 + 512],
                    in0=un[0:64, 0:512],
                    in1=rb0[:, :],
                )
                if uhi is not None:
                    rb1 = rbsp.tile([128, 512], bf, tag="rbs", name=f"rb1_{pid}")
                    nc.gpsimd.partition_broadcast(
                        rb1[64:128, :], rcp[p1 : p1 + 1, f1 : f1 + 512]
                    )
                    nc.vector.tensor_mul(
                        out=aoT_sb[64:128, jj, qq0 : qq0 + 512],
                        in0=uhi[64:128, :],
                        in1=rb1[64:128, :],
                    )
                    return
                rb1 = rbsp.tile([64, 512], bf, tag="rbs", name=f"rb1_{pid}")
                nc.gpsimd.partition_broadcast(
                    rb1[:, :], rcp[p1 : p1 + 1, f1 : f1 + 512]
                )
                t1 = tmpp.tile([64, 512], bf, tag="t1", name=f"t1_{pid}")
                nc.vector.tensor_mul(
                    out=t1[:, :], in0=un[0:64, 512:1024], in1=rb1[:, :]
                )
                nc.sync.dma_start(aoT_sb[64:128, jj, qq0 : qq0 + 512], t1[:, :])

        def emit_proj(qc):
            # partial projection (contract the core's CH channels) for one
            # 128-row chunk
            if LEVEL < 4:
                return
            with nc.named_scope("proj"):
                y_sb = ypool.tile([128, C], bf, tag="y", name=f"y_{qc}")
                q0 = qc * 128
                for nt in range(2):
                    pj = ps_pj.tile([128, 512], f32, tag="pj", name=f"pj_{qc}_{nt}")
                    for ci in range(PAIRS):
                        nc.tensor.matmul(
                            out=pj[:, :],
                            lhsT=aoT_sb[:, ci, q0 : q0 + 128],
                            rhs=wt_sb[:, ci, nt * 512 : (nt + 1) * 512],
                            start=(ci == 0),
                            stop=(ci == PAIRS - 1),
                        )
                    nc.vector.tensor_copy(
                        y_sb[:, nt * 512 : (nt + 1) * 512], pj[:, :]
                    )
                if TUNE["proj_dma"]:
                    nc.sync.dma_start(yp_d[q0 : q0 + 128, :], y_sb[:, :])

        def emit_proj_tail(chunks, interleave):
            # Tail projections in ci-waves over 2-chunk groups on the (now
            # idle) 2-bank sc pool tiles.  The ci=0/1 matmuls only need the
            # pairs normalized mid-stream, so the PE fills the window while
            # the last pairs' normalizations (interleave callbacks) land.
            if LEVEL < 4:
                return
            groups = [chunks[i : i + 2] for i in range(0, len(chunks), 2)]
            first = True
            for grp in groups:
                with nc.named_scope("proj"):
                    pjts = {
                        qc: ps_sc.tile([128, 2, 512], f32, tag="sc", name=f"pj_{qc}")
                        for qc in grp
                    }
                    for ci in range(PAIRS):
                        if first and ci in interleave:
                            interleave.pop(ci)()
                        for qc in grp:
                            for nt in range(2):
                                nc.tensor.matmul(
                                    out=pjts[qc][:, nt, :],
                                    lhsT=aoT_sb[:, ci, qc * 128 : qc * 128 + 128],
                                    rhs=wt_sb[:, ci, nt * 512 : (nt + 1) * 512],
                                    start=(ci == 0),
                                    stop=(ci == PAIRS - 1),
                                )
                    first = False
                    for qc in grp:
                        # ACT is idle in the tail: split copies across engines
                        # and DMA each half as soon as its copy lands
                        y_sb = ypool.tile([128, C], bf, tag="y", name=f"y_{qc}")
                        nc.scalar.copy(y_sb[:, 0:512], pjts[qc][:, 0, :])
                        if TUNE["proj_dma"]:
                            nc.sync.dma_start(
                                yp_d[qc * 128 : qc * 128 + 128, 0:512],
                                y_sb[:, 0:512],
                            )
                        nc.vector.tensor_copy(y_sb[:, 512:1024], pjts[qc][:, 1, :])
                        if TUNE["proj_dma"]:
                            nc.sync.dma_start(
                                yp_d[qc * 128 : qc * 128 + 128, 512:1024],
                                y_sb[:, 512:1024],
                            )

        pending_proj = []

        # Optional benchmark loop: repeat the whole compute body loop_n
        # times in one NEFF (the body is idempotent) to measure HW time as
        # a wall-clock delta without per-call transfer overhead.
        loop_ctx = (
            tc.For_i(0, TUNE["loop_n"], 1) if TUNE["loop_n"] > 1 else None
        )
        if loop_ctx is not None:
            ctx.enter_context(loop_ctx)

        # Flat software-pipelined stream over (pair, kb): scores+exp lead,
        # attn@V trails by pipe_depth tiles (crossing pair boundaries), the
        # normalize broadcast trails one batch, projection trails one q-tile.
        DEPTH = TUNE["pipe_depth"]
        NB_DELAY = TUNE["nb_delay"]
        PP_EVERY = TUNE["pp_every"]
        normed = []  # (tick, norm_b payload) awaiting emission
        pending_act = []  # (tick, batched Ln/exp closure) awaiting emission
        ACT_DELAY = TUNE["act_delay"]
        tick = [0]
        for qt in range(4):
            q0 = qt * 512
            nkb = 4 * qt + 4
            accs = {}
            uns = {}
            pends = []       # (j, kb, c0, at)
            done_pairs = []  # pairs whose last attn@V has been emitted
            # per-2-pair-batch staging for softmax sums / reciprocals: the 4
            # sum rows (2 pairs x 2 heads) sit at partitions {0,32,64,96} (the
            # only legal SBUF single-partition bases), so one full-height ACT
            # op batches the Ln/exp and partition_broadcast can read each row.
            batch_st = {}  # batch index -> (srow, rcp)

            def pop_av():
                pj_, pkb, pc0, pat = pends.pop(0)
                pacc = accs[pj_]
                if LEVEL >= 2:
                    with nc.named_scope("av"):
                        for h in range(2):
                            nc.tensor.matmul(
                                out=pacc[h][0:65, pc0:],
                                lhsT=v_sb[
                                    :, pkb, (2 * pj_ + h) * 65 : (2 * pj_ + h) * 65 + 65
                                ],
                                rhs=pat[:, h, pc0:],
                                start=(pkb == 0),
                                stop=(pkb == nkb - 1),
                            )
                if pkb == nkb - 1:
                    done_pairs.append(pj_)

            def flush_done():
                # norm_a for any pair whose attn@V finished: copy the
                # unnormalized outputs (rows 0-64, incl. the sum row) to
                # SBUF to release the acc banks, then DMA-gather the two
                # sum rows onto the staging partitions.  Once the last pair
                # of a batch group is in, run the batched Ln + exp(-x).
                while done_pairs:
                    dj = done_pairs.pop(0)
                    dacc = accs.pop(dj)
                    if LEVEL < 3:
                        continue
                    direct = TUNE["tail_direct_norm"] and qt == 3 and dj >= 2
                    if direct:
                        # tail: no batching partner worth waiting for -- take
                        # the Ln straight off the PSUM sum rows (partition 64
                        # is a legal base) and skip the gather DMAs
                        with nc.named_scope("norm"):
                            lnt = rcpp.tile(
                                [128, 1024], f32, tag="rcp", name=f"lnt_{qt}_{dj}"
                            )
                            rcp = rcpp.tile(
                                [128, 1024], bf, tag="rcp", name=f"rcp_{qt}_{dj}"
                            )
                            for h in range(2):
                                nc.scalar.activation(
                                    out=lnt[64:65, h * 512 : (h + 1) * 512],
                                    in_=dacc[h][64:65, :],
                                    func=Ln,
                                )
                            nc.scalar.activation(
                                out=rcp[64:65, :],
                                in_=lnt[64:65, :],
                                func=Exp,
                                scale=-1.0,
                            )
                            un = sums.tile(
                                [65, 1024], bf, tag="un", name=f"un_{qt}_{dj}"
                            )
                            nc.vector.tensor_copy(un[:, 0:512], dacc[0][0:65, :])
                            nc.vector.tensor_copy(un[:, 512:1024], dacc[1][0:65, :])
                            # park the h1 half at partitions 64-127 now (PE
                            # identity move into a freed acc bank, then DVE
                            # copy to SBUF) so the normalize never needs a
                            # cross-partition DMA on the critical path
                            uhi_ps = ps_ac.tile(
                                [128, 512], f32, tag="acc", name=f"uhp_{qt}_{dj}"
                            )
                            nc.tensor.matmul(
                                out=uhi_ps[64:128, :],
                                lhsT=id_sb[:, :],
                                rhs=un[0:64, 512:1024],
                                start=True,
                                stop=True,
                            )
                            if TUNE["rb_pbcast"]:
                                uhi = uhi_ps
                            else:
                                uhi = sums.tile(
                                    [128, 512], bf, tag="un", name=f"uhs_{qt}_{dj}"
                                )
                                nc.vector.tensor_copy(
                                    uhi[64:128, :], uhi_ps[64:128, :]
                                )
                        normed.append(
                            (
                                tick[0],
                                (
                                    dj,
                                    q0,
                                    un,
                                    rcp,
                                    [(64, 0), (64, 512)],
                                    uhi,
                                    f"{qt}_{dj}",
                                ),
                            )
                        )
                        continue
                    bi = dj // 2
                    if dj % 2 == 0:
                        srow = srowp.tile(
                            [128, 512], bf, tag="srow", name=f"srow_{qt}_{bi}"
                        )
                        # unwritten partitions must stay finite (and owned by
                        # this tile generation) for the full-height Ln below
                        nc.vector.memset(srow[:, :], 1.0)
                        batch_st[bi] = srow
                    srow = batch_st[bi]
                    with nc.named_scope("norm"):
                        un = sums.tile([65, 1024], bf, tag="un", name=f"un_{qt}_{dj}")
                        nc.vector.tensor_copy(un[:, 0:512], dacc[0][0:65, :])
                        nc.vector.tensor_copy(un[:, 512:1024], dacc[1][0:65, :])
                        uns[dj] = un
                        for h in range(2):
                            # 1-partition "broadcast" = cross-partition copy
                            # on the Pool engine; no DMA-queue latency
                            p0 = 64 * (dj % 2) + 32 * h
                            if TUNE["gather_pbcast"]:
                                nc.gpsimd.partition_broadcast(
                                    srow[p0 : p0 + 1, :],
                                    un[64:65, h * 512 : (h + 1) * 512],
                                )
                            else:
                                nc.sync.dma_start(
                                    srow[p0 : p0 + 1, :],
                                    un[64:65, h * 512 : (h + 1) * 512],
                                )
                    if dj % 2 == 1:
                        # defer the batched Ln/exp emission into the next few
                        # stream ticks: it waits on the gather DMAs, and the
                        # ACT queue is strict FIFO -- emitted here it would
                        # block the following exps behind that DMA latency
                        def batch_act(dj=dj, bi=bi, srow=srow, qq0=q0, myqt=qt):
                            with nc.named_scope("norm"):
                                lnt = rcpp.tile(
                                    [128, 512], f32, tag="rcp", name=f"lnt_{myqt}_{bi}"
                                )
                                rcp = rcpp.tile(
                                    [128, 512], bf, tag="rcp", name=f"rcp_{myqt}_{bi}"
                                )
                                nc.scalar.activation(
                                    out=lnt[:, :], in_=srow[:, :], func=Ln
                                )
                                nc.scalar.activation(
                                    out=rcp[:, :],
                                    in_=lnt[:, :],
                                    func=Exp,
                                    scale=-1.0,
                                )
                            for bj in (dj - 1, dj):
                                r0 = 64 * (bj % 2)
                                normed.append(
                                    (
                                        tick[0],
                                        (
                                            bj,
                                            qq0,
                                            uns.pop(bj),
                                            rcp,
                                            [(r0, 0), (r0 + 32, 0)],
                                            None,
                                            f"{myqt}_{bj}",
                                        ),
                                    )
                                )

                        pending_act.append((tick[0], batch_act))

            for j in range(PAIRS):
                accs[j] = [
                    ps_ac.tile([128, 512], f32, tag="acc", name=f"acc0_{qt}_{j}"),
                    ps_ac.tile([128, 512], f32, tag="acc", name=f"acc1_{qt}_{j}"),
                ]
                for kb in range(nkb):
                    c0 = max(0, (kb - 4 * qt) * 128)
                    sc = ps_sc.tile(
                        [128, 2, 512], f32, tag="sc", name=f"sc_{qt}_{j}_{kb}"
                    )
                    with nc.named_scope("sc"):
                        for h in range(2):
                            nc.tensor.matmul(
                                out=sc[:, h, c0:],
                                lhsT=kT_sb[
                                    h * 64 : (h + 1) * 64, j, kb * 128 : (kb + 1) * 128
                                ],
                                rhs=qT_sb[h * 64 : (h + 1) * 64, j, q0 + c0 : q0 + 512],
                                start=True,
                                stop=True,
                            )
                    at = atp.tile([128, 2, 512], bf, tag="at", name=f"at_{qt}_{j}_{kb}")
                    if LEVEL >= 1:
                        with nc.named_scope("exp"):
                            nc.scalar.activation(
                                out=at[:, :, c0:],
                                in_=sc[:, :, c0:],
                                func=Exp,
                                scale=SCALE,
                            )
                        if kb >= 4 * qt:  # diagonal block: causal staircase mask
                            eng = nc.gpsimd if TUNE["mask_gpsimd"] else nc.vector
                            with nc.named_scope("mask"):
                                eng.tensor_mul(
                                    out=at[:, :, c0 : c0 + 128],
                                    in0=at[:, :, c0 : c0 + 128],
                                    in1=mk_v,
                                )
                    pends.append((j, kb, c0, at))
                    if len(pends) > DEPTH:
                        pop_av()
                        flush_done()
                    tick[0] += 1
                    # interleave deferred work from previous batches/q-tiles
                    if pending_act and tick[0] - pending_act[0][0] >= ACT_DELAY:
                        pending_act.pop(0)[1]()
                    if normed and tick[0] - normed[0][0] >= NB_DELAY:
                        norm_b(normed.pop(0)[1])
                    if (
                        TUNE["proj_interleave"]
                        and pending_proj
                        and tick[0] % PP_EVERY == 0
                        and qt > 0
                    ):
                        emit_proj(pending_proj.pop(0))

            while pends:
                pop_av()
            flush_done()
            if qt == 3:
                while pending_act:
                    pending_act.pop(0)[1]()
            pending_proj.extend(range(4 * qt, 4 * qt + 4))
            if qt == 3:
                # tail: the not-yet-normalized pairs' norm_b calls slot in
                # just before the first ci-wave that reads their aoT rows
                late = [p for _, p in normed]
                normed.clear()
                interleave = {
                    (PAIRS - len(late) + i): (lambda pp=pp: norm_b(pp))
                    for i, pp in enumerate(late)
                }
                if TUNE["tail_sc_proj"]:
                    emit_proj_tail(pending_proj, interleave)
                    pending_proj.clear()
                else:
                    for pp in late:
                        norm_b(pp)
                    while pending_proj:
                        emit_proj(pending_proj.pop(0))

    nc.finalize()
    return nc


def _get_program():
    if "nc" not in _CACHED:
        _CACHED["nc"] = _build_program()
    return _CACHED["nc"]


def _prep_inputs(q, k, v, W_out):
    """Build the 8 per-core input maps (host-side shard + transpose + cast)."""
    mk = np.zeros((128, 2, 128), np.float32)
    kk = np.arange(128)[:, None]
    oo = np.arange(128)[None, :]
    mk[:, 0, :] = (kk <= oo).astype(np.float32)
    mk[:, 1, :] = mk[:, 0, :]
    mk = mk.reshape(128, 256).astype(bf16)
    id64 = np.eye(64, dtype=np.float32).astype(bf16)
    ones = np.ones((128, 64), bf16)

    in_maps = []
    for core in range(NCORES):
        b, hh = core // 2, core % 2
        ch0 = hh * CH
        qT = np.ascontiguousarray(q[b].T[ch0 : ch0 + CH]).astype(bf16)
        kT = np.ascontiguousarray(k[b].T[ch0 : ch0 + CH]).astype(bf16)
        vh = v[b].reshape(T, H, HD)[:, hh * HPC : (hh + 1) * HPC, :]
        vx = np.concatenate(
            [vh.astype(np.float32), np.ones((T, HPC, 1), np.float32)], axis=2
        )
        vx = np.ascontiguousarray(vx.reshape(T, HPC * 65)).astype(bf16)
        wT = np.ascontiguousarray(W_out.T[ch0 : ch0 + CH]).astype(bf16)
        in_maps.append(
            {
                "qT": qT,
                "kT": kT,
                "vx": vx,
                "wT": wT,
                "mk": mk,
                "id64": id64,
                "on": ones,
            }
        )
    return in_maps


def _run(in_maps, trace=False):
    from concourse.bass_utils import run_bass_kernel_spmd

    nc = _get_program()
    return run_bass_kernel_spmd(
        nc, in_maps, core_ids=list(range(NCORES)), trace=trace
    )


def kernel(q, k, v, W_out, b_out, _trace=False, _return_res=False):
    q = np.asarray(q, np.float32)
    k = np.asarray(k, np.float32)
    v = np.asarray(v, np.float32)
    W_out = np.asarray(W_out, np.float32)
    b_out = np.asarray(b_out, np.float32)

    in_maps = _prep_inputs(q, k, v, W_out)
    res = _run(in_maps, trace=_trace)

    y = np.empty((B, T, C), np.float32)
    for b in range(B):
        y[b] = res.results[2 * b]["yp"].astype(np.float32) + res.results[2 * b + 1][
            "yp"
        ].astype(np.float32)
    y += b_out[None, None, :]
    if _return_res:
        return y, res
    return y
